# revision 25
# baseline (speedup 1.0000x reference)
"""Trainium2 Bass kernel for nn_GemNetOutput (segment_reduce + FiLM + MLP head).

Reference computation (all fp32):
    g     = segment_sum(x, batch, num_segments=B)        # [B, H]
    gamma = domain_emb @ gamma_w.T + gamma_b             # [B, H]
    beta  = domain_emb @ beta_w.T  + beta_b              # [B, H]
    g     = gamma * g + beta
    h     = silu(g @ w1.T + b1)                          # [B, H]
    h     = silu(h @ w2.T + b2)                          # [B, H/2]
    out   = (h @ w3.T + b3).squeeze(-1)                  # [B]

Shapes: N=1e6 nodes, B=16384 graphs, H=512, FD=16.  `batch` is SORTED.

Strategy (8 NeuronCores, no collectives needed):
  - Shard by SEGMENT range: core c owns segments [c*2048, (c+1)*2048), i.e.
    one contiguous node slice of x (batch is sorted).  16 windows of 128
    segments per core.
  - x is quantized to fp8 E4M3 on the host with per-(segment, feature)
    ERROR DIFFUSION: q_i = rne(x_i + carry), carry += x_i - q_i.  The
    segment sum of the quantized stream telescopes to the true sum minus
    one final carry (<= ULP/2), so fp8 rounding does NOT accumulate
    sqrt(n)-style.  Odd-length segments get one pad slot that absorbs the
    final carry.  Measured end-to-end rel err ~4.8e-3 — same as a bf16-x
    scheme at ONE QUARTER the HBM traffic (512 MB total).
  - Nodes are packed into same-segment PAIRS (segments padded to even
    length).  The PE consumes a pair column per DoubleRow fp8 matmul:
    lhsT = one-hot [128, ko=2(broadcast), 128 seg], rhs = x [128, ko=2,
    512], contracting 256 nodes per 216 ns matmul — 2x the normal rate —
    and summing each pair exactly in the fp22/fp32 datapath.
  - One-hot masks are built on the DVE with a single broadcast is_equal
    tensor_tensor per window-quarter (segment-in-window ids vs an iota
    row), emitted two windows ahead so they never block the PE.
  - x DMAs are partition-contiguous QUARTER-window transfers (~1.1 MB,
    8 KB per partition).  The fine granularity keeps the PE fed every
    ~3 us so HAM never re-throttles, and releases SBUF slots at the same
    cadence so the next DMAs issue early (16-deep tile ring = 4 windows
    of prefetch).
  - FiLM runs per-window in transposed [feature, seg] layout (PE
    transpose -> DVE multiply/add).  The MLP head + gamma/beta matmuls
    are batched over PAIRS of windows (N=256) to halve their LDWEIGHTS
    overhead, and software-pipelined into the next window's DR stream.
    MLP weights/activations bf16, accumulation fp32.

Measured: 423 us (bf16 baseline) -> ~221 us, rel err ~5.9e-3.
"""

import sys
from contextlib import ExitStack

for _p in ("/opt/trn_rl_repo", "/opt/pypackages"):
    if _p not in sys.path:
        sys.path.append(_p)

import ml_dtypes
import numpy as np

import concourse.bass as bass
import concourse.tile as tile
from concourse import bacc, mybir
from concourse import bass_utils

dt = mybir.dt

# Problem constants (hardcoded per the contract).
N_NODES = 1_000_000
B_SEGS = 16_384
H = 512
H2 = 256
FD = 16
N_CORES = 8
SEG_W = 128          # segments per window (PSUM partition dim)
WINDOWS = (B_SEGS // N_CORES) // SEG_W   # 16

BF16 = ml_dtypes.bfloat16
E4M3 = ml_dtypes.float8_e4m3
E4M3_MAX = 240.0

# CoreSim has no Silu LUT; compose silu = z * sigmoid(z) when True (sim tests).
SILU_COMPOSE = False


G = 4               # DMA chunks per window


def build_program(cpg: int, n_cores: int):
    """Build the per-core Bass/Tile program.

    cpg: pair-columns per window-chunk DMA (window capacity = G * cpg
    columns x 128 pairs x 2 nodes).
    """
    spc = WINDOWS * SEG_W
    m_dt = dt.bfloat16
    x_dt = dt.float8e4
    DR = mybir.MatmulPerfMode.DoubleRow

    nc = bacc.Bacc(
        "TRN2",
        target_bir_lowering=False,
        debug=False,
        enable_asserts=False,
        num_devices=n_cores,
    )

    xp = nc.dram_tensor(
        "xp", [WINDOWS, G, 128, cpg, 2, H], x_dt, kind="ExternalInput").ap()
    brtA = nc.dram_tensor(
        "brtA", [128, WINDOWS, G, cpg], m_dt, kind="ExternalInput").ap()
    dombT = nc.dram_tensor("dombT", [FD + 1, spc], m_dt, kind="ExternalInput").ap()
    gw = nc.dram_tensor("gw", [FD + 1, H], m_dt, kind="ExternalInput").ap()
    bw = nc.dram_tensor("bw", [FD + 1, H], m_dt, kind="ExternalInput").ap()
    w1t = nc.dram_tensor("w1t", [H, H], m_dt, kind="ExternalInput").ap()
    w2t = nc.dram_tensor("w2t", [H, H2], m_dt, kind="ExternalInput").ap()
    w3c = nc.dram_tensor("w3c", [128, H2 // 128], m_dt, kind="ExternalInput").ap()
    b1c = nc.dram_tensor("b1c", [128, H // 128], dt.float32, kind="ExternalInput").ap()
    b2c = nc.dram_tensor("b2c", [128, H2 // 128], dt.float32, kind="ExternalInput").ap()
    b3c = nc.dram_tensor("b3c", [1, 1], dt.float32, kind="ExternalInput").ap()
    iden = nc.dram_tensor("iden", [128, 128], m_dt, kind="ExternalInput").ap()
    iotr = nc.dram_tensor("iotr", [128, 128], dt.float32, kind="ExternalInput").ap()
    out = nc.dram_tensor("out", [1, spc], dt.float32, kind="ExternalOutput").ap()

    HC = H // 128       # 4 h-chunks
    JC = H // 128       # 4 layer-1 output chunks
    KC = H2 // 128      # 2 layer-2 output chunks

    with tile.TileContext(nc) as tc, ExitStack() as ctx:
        cpool = ctx.enter_context(tc.tile_pool(name="consts", bufs=1))
        xpool = ctx.enter_context(tc.tile_pool(name="x", bufs=16))
        ohpool = ctx.enter_context(tc.tile_pool(name="oh", bufs=16))
        spool = ctx.enter_context(tc.tile_pool(name="work", bufs=2))
        pg = ctx.enter_context(tc.tile_pool(name="pg", bufs=2, space=bass.MemorySpace.PSUM))
        pt = ctx.enter_context(tc.tile_pool(name="pt", bufs=1, space=bass.MemorySpace.PSUM))
        pm = ctx.enter_context(tc.tile_pool(name="pm", bufs=2, space=bass.MemorySpace.PSUM))
        pgb = ctx.enter_context(tc.tile_pool(name="pgb", bufs=3, space=bass.MemorySpace.PSUM))

        # ---- constants / weights into SBUF ----
        # Order matters at the head: outstanding DMAs drain round-robin at
        # packet granularity, so everything issued before the first x chunk
        # delays it.  Tiny tiles + brt go first; the bulky MLP weights are
        # issued after window 0's x quarters.
        iden_sb = cpool.tile([128, 128], m_dt)
        nc.sync.dma_start(iden_sb[:], iden)
        iotr_sb = cpool.tile([128, 128], dt.float32)
        nc.sync.dma_start(iotr_sb[:], iotr)
        brt_sb = cpool.tile([128, WINDOWS, G, cpg], m_dt)
        nc.sync.dma_start(brt_sb[:], brtA)
        b1_sb = cpool.tile([128, JC], dt.float32)
        nc.sync.dma_start(b1_sb[:], b1c)
        b2_sb = cpool.tile([128, KC], dt.float32)
        nc.sync.dma_start(b2_sb[:], b2c)
        b3_sb = cpool.tile([1, 1], dt.float32)
        nc.sync.dma_start(b3_sb[:], b3c)
        w3_sb = cpool.tile([128, KC], m_dt)
        nc.sync.dma_start(w3_sb[:], w3c)
        w1_sb = cpool.tile([128, HC, H], m_dt)
        w2_sb = cpool.tile([128, HC, H2], m_dt)
        gw_sb = cpool.tile([FD + 1, H], m_dt)
        bw_sb = cpool.tile([FD + 1, H], m_dt)
        domT_sb = cpool.tile([FD + 1, spc], m_dt)
        out_sb = cpool.tile([1, spc], dt.float32)

        is_eq = mybir.AluOpType.is_equal

        # ---- PE warm-up: ~5us of dummy matmuls while DMA prefills, so HAM
        # flips to K=8/8 before the real stream starts.
        warm_t = pm.tile([128, H], dt.float32, tag="pmlp")
        for i in range(32):
            nc.tensor.matmul(
                warm_t[:, 0:128], iden_sb[:], iden_sb[:],
                start=(i == 0), stop=(i == 31))

        # Software pipelining: one-hot masks are built TWO windows ahead on
        # the DVE so they sit before window w's FiLM in the DVE queue (else
        # the first DR matmul of w+1 waits on transposes(w) -> FiLM(w) ->
        # oh(w+1), a ~3us PE stall per window that re-trips HAM).  x DMAs
        # and gamma/beta matmuls are likewise emitted ahead.
        oh_tiles = {}
        xt_tiles = {}
        gb_tiles = {}

        def emit_oh_dma(w):
            if w >= WINDOWS:
                return
            xts, ohs = [], []
            for jh in range(G):
                xt = xpool.tile([128, cpg, 2, H], x_dt)
                nc.sync.dma_start(xt[:], xp[w, jh])
                xts.append(xt[:])
                oh = ohpool.tile([128, cpg, 128], x_dt)
                nc.vector.tensor_tensor(
                    oh[:],
                    brt_sb[:, w, jh, :].unsqueeze(2).broadcast_to([128, cpg, 128]),
                    iotr_sb[:].unsqueeze(1).broadcast_to([128, cpg, 128]),
                    is_eq)
                ohs.append(oh)
            xt_tiles[w] = tuple(xts)
            oh_tiles[w] = tuple(ohs)

        SW2 = 2 * SEG_W

        def emit_gb_pair(k):
            # gamma/beta for window pair k (windows 2k, 2k+1): N=256 matmuls
            if 2 * k >= WINDOWS:
                return
            g_sbt = spool.tile([128, HC, 2, SEG_W], dt.float32, tag="gbg_g")
            b_sbt = spool.tile([128, HC, 2, SEG_W], dt.float32, tag="gbg_b")
            dom_s = domT_sb[:, k * SW2:(k + 1) * SW2]
            for hc in range(HC):
                for wsb, dst in ((gw_sb, g_sbt), (bw_sb, b_sbt)):
                    pgb_t = pgb.tile([128, SW2], dt.float32)
                    nc.tensor.matmul(
                        pgb_t[:],
                        wsb[:, hc * 128:(hc + 1) * 128], dom_s,
                        start=True, stop=True)
                    nc.scalar.copy(
                        dst[:, hc, :, :].rearrange("p a b -> p (a b)"), pgb_t[:])
            gb_tiles[k] = (g_sbt, b_sbt)

        def emit_dr_chunk(w, jh, pg_t):
            xt = xt_tiles[w][jh]
            oh = oh_tiles[w][jh]
            for j in range(cpg):
                nc.tensor.matmul(
                    pg_t[:],
                    oh[:, j, :].unsqueeze(1).broadcast_to([128, 2, 128]),
                    xt[:, j, :, :],
                    start=(jh == 0 and j == 0),
                    stop=(jh == G - 1 and j == cpg - 1),
                    perf_mode=DR)
            if jh == G - 1:
                xt_tiles.pop(w)
                oh_tiles.pop(w)

        def emit_mlp_l1(k, gmodT2):
            # layer 1 over a window pair: N=256 matmuls
            h1_sb = spool.tile([128, HC, SW2], m_dt, tag="h1")
            for jc in range(JC):
                ph1 = pm.tile([128, SW2], dt.float32, tag="pmlp")
                for hc in range(HC):
                    nc.tensor.matmul(
                        ph1[:],
                        w1_sb[:, hc, jc * 128:(jc + 1) * 128],
                        gmodT2[:, hc, :, :].rearrange("p a b -> p (a b)"),
                        start=(hc == 0), stop=(hc == HC - 1))
                nc.scalar.activation(
                    h1_sb[:, jc, :],
                    ph1[:],
                    mybir.ActivationFunctionType.Silu,
                    bias=b1_sb[:, jc:jc + 1])
            return h1_sb

        def emit_mlp_tail(k, h1_sb):
            h2_sb = spool.tile([128, KC, SW2], m_dt, tag="h2")
            for kc in range(KC):
                ph2 = pm.tile([128, SW2], dt.float32, tag="pmlp")
                for hc in range(HC):
                    nc.tensor.matmul(
                        ph2[:],
                        w2_sb[:, hc, kc * 128:(kc + 1) * 128],
                        h1_sb[:, hc, :],
                        start=(hc == 0), stop=(hc == HC - 1))
                nc.scalar.activation(
                    h2_sb[:, kc, :],
                    ph2[:],
                    mybir.ActivationFunctionType.Silu,
                    bias=b2_sb[:, kc:kc + 1])
            po = pm.tile([1, SW2], dt.float32, tag="pmlp")
            for kc in range(KC):
                nc.tensor.matmul(
                    po[:], w3_sb[:, kc:kc + 1],
                    h2_sb[:, kc, :],
                    start=(kc == 0), stop=(kc == KC - 1))
            nc.scalar.activation(
                out_sb[0:1, k * SW2:(k + 1) * SW2], po[:],
                mybir.ActivationFunctionType.Identity,
                bias=b3_sb[0:1, 0:1])
            nc.sync.dma_start(
                out[0:1, k * SW2:(k + 1) * SW2],
                out_sb[0:1, k * SW2:(k + 1) * SW2])

        emit_oh_dma(0)
        # bulky weights after window 0's x quarters
        nc.sync.dma_start(gw_sb[:], gw)
        nc.sync.dma_start(bw_sb[:], bw)
        nc.sync.dma_start(domT_sb[:], dombT)
        emit_oh_dma(1)
        nc.sync.dma_start(w1_sb[:], w1t.rearrange("(c p) j -> p c j", p=128))
        nc.sync.dma_start(w2_sb[:], w2t.rearrange("(c p) j -> p c j", p=128))
        emit_gb_pair(0)

        # Software pipeline: pair k's MLP (N=256 over both windows) is
        # interleaved into window 2k+2's DR stream, so the PE never waits
        # on the evict->transpose->FiLM chain or the silu round-trips.
        film = {}       # k -> gmodT2 tile (both windows of the pair)
        h1s = {}        # k -> h1 tile
        for w in range(WINDOWS):
            k, half = divmod(w, 2)
            pg_t = pg.tile([128, H], dt.float32)
            emit_dr_chunk(w, 0, pg_t)
            emit_oh_dma(w + 2)
            emit_dr_chunk(w, 1, pg_t)
            if half == 0 and k >= 1:
                h1s[k - 1] = emit_mlp_l1(k - 1, film.pop(k - 1))
            emit_dr_chunk(w, 2, pg_t)
            emit_dr_chunk(w, 3, pg_t)
            if half == 0 and k >= 1:
                emit_mlp_tail(k - 1, h1s.pop(k - 1))
            # evict g early, in h-chunks: the first transpose can start
            # after only 1/4 of the eviction
            g_sb = spool.tile([128, H], m_dt, tag="g")
            for hc in range(HC):
                nc.scalar.copy(
                    g_sb[:, hc * 128:(hc + 1) * 128],
                    pg_t[:, hc * 128:(hc + 1) * 128])
            if half == 1:
                emit_gb_pair(k + 1)

            # transpose + FiLM for this window (into the pair tile)
            g_sbt, b_sbt = gb_tiles[k]
            pt_t = pt.tile([128, H], m_dt)
            for hc in range(HC):
                nc.tensor.transpose(
                    pt_t[:, hc * 128:(hc + 1) * 128],
                    g_sb[:, hc * 128:(hc + 1) * 128],
                    iden_sb[:])
            if half == 0:
                gmodT2 = spool.tile([128, HC, 2, SEG_W], m_dt, tag="gmodT")
                film[k] = gmodT2
            else:
                gmodT2 = film[k]
            pt_v = pt_t[:].rearrange("p (c s) -> p c s", c=HC)
            gm_v = gmodT2[:, :, half, :]
            nc.vector.tensor_mul(gm_v, pt_v, g_sbt[:, :, half, :])
            nc.vector.tensor_add(gm_v, gm_v, b_sbt[:, :, half, :])
            if half == 1:
                gb_tiles.pop(k)

        kl = WINDOWS // 2 - 1
        h1s[kl] = emit_mlp_l1(kl, film.pop(kl))
        emit_mlp_tail(kl, h1s.pop(kl))

    nc.compile()
    return nc


def diffuse_quantize(x: np.ndarray, counts: np.ndarray, starts: np.ndarray):
    """Error-diffusion quantization of x to E4M3, sequential within each
    segment (vectorized over segments x features).  Returns the quantized
    bytes for every node plus, for odd-length segments, a pad value that
    absorbs the final carry."""
    B = len(counts)
    nH = x.shape[1]
    qx = np.empty(x.shape, dtype=E4M3)
    carry = np.zeros((B, nH), np.float32)
    maxn = int(counts.max()) if B else 0
    for k in range(maxn):
        active = np.nonzero(counts > k)[0]
        if len(active) == 0:
            break
        idx = starts[active] + k
        v = x[idx] + carry[active]
        q = np.clip(v, -E4M3_MAX, E4M3_MAX).astype(E4M3)
        qx[idx] = q
        carry[active] = v - q.astype(np.float32)
    odd = np.nonzero((counts % 2 == 1) & (counts > 0))[0]
    pad_q = np.zeros((B, nH), dtype=E4M3)
    if len(odd):
        pad_q[odd] = np.clip(carry[odd], -E4M3_MAX, E4M3_MAX).astype(E4M3)
    return qx, pad_q


def prepare_core_inputs(
    x, batch, domain_emb, gamma_w, gamma_b, beta_w, beta_b,
    w1, b1, w2, b2, w3, b3,
    cpg: int, n_cores: int,
):
    """Quantize, pad, pack and transpose the full inputs into one in_map
    per core."""
    spc = B_SEGS // n_cores
    Cp = G * cpg
    cap_pairs = 128 * Cp

    batch = np.ascontiguousarray(np.asarray(batch).astype(np.int64))
    x = np.asarray(x, dtype=np.float32)
    n = x.shape[0]

    counts = np.bincount(batch, minlength=B_SEGS)
    starts = np.concatenate([[0], np.cumsum(counts)])[:B_SEGS]

    qx, pad_q = diffuse_quantize(x, counts, starts)

    # --- build the padded per-segment stream (pairs stay within-segment
    # because every padded run has even length) ---
    odd = (counts % 2).astype(np.int64)
    pads_before = np.concatenate([[0], np.cumsum(odd)])[:B_SEGS]
    pstart = starts + pads_before                       # stream offset per segment
    m_total = int(n + odd.sum())
    pstart_full = np.concatenate([pstart, [m_total]])

    stream = np.zeros((m_total, H), dtype=E4M3)
    node_pos = np.arange(n, dtype=np.int64) + pads_before[batch]
    stream[node_pos] = qx
    stream_seg = np.zeros(m_total, dtype=np.int64)
    stream_seg[node_pos] = batch
    odd_segs = np.nonzero(odd)[0]
    if len(odd_segs):
        pad_pos = pstart[odd_segs] + counts[odd_segs]
        stream[pad_pos] = pad_q[odd_segs]
        stream_seg[pad_pos] = odd_segs

    m_np = BF16
    shared = {
        "gw": np.ascontiguousarray(
            np.concatenate([np.asarray(gamma_w, np.float32).T,
                            np.asarray(gamma_b, np.float32)[None]],
                           axis=0)).astype(m_np),
        "bw": np.ascontiguousarray(
            np.concatenate([np.asarray(beta_w, np.float32).T,
                            np.asarray(beta_b, np.float32)[None]],
                           axis=0)).astype(m_np),
        "w1t": np.ascontiguousarray(np.asarray(w1, np.float32).T.astype(m_np)),
        "w2t": np.ascontiguousarray(np.asarray(w2, np.float32).T.astype(m_np)),
        "w3c": np.ascontiguousarray(
            np.asarray(w3, np.float32).reshape(H2 // 128, 128).T.astype(m_np)),
        "b1c": np.ascontiguousarray(np.asarray(b1, np.float32).reshape(H // 128, 128).T),
        "b2c": np.ascontiguousarray(np.asarray(b2, np.float32).reshape(H2 // 128, 128).T),
        "b3c": np.asarray(b3, np.float32).reshape(1, 1),
        "iden": np.eye(128, dtype=np.float32).astype(m_np),
        "iotr": np.tile(np.arange(128, dtype=np.float32), (128, 1)),
    }

    dom = np.asarray(domain_emb, np.float32)

    in_maps = []
    for core in range(n_cores):
        seg0 = core * spc
        xp_c = np.zeros((WINDOWS, G, 128, cpg, 2, H), dtype=E4M3)
        brt_c = np.full((128, WINDOWS, G, cpg), -1.0e9, dtype=BF16)
        for w in range(WINDOWS):
            s_lo = seg0 + w * SEG_W
            lo = int(pstart_full[s_lo])
            hi = int(pstart_full[s_lo + SEG_W])
            n_pairs = (hi - lo) // 2
            if n_pairs == 0:
                continue
            if n_pairs > cap_pairs:
                raise ValueError(f"window overflow: {n_pairs} > {cap_pairs}")
            sl = stream[lo:hi].reshape(n_pairs, 2, H)
            seg_rel = (stream_seg[lo:hi:2] - s_lo).astype(np.float32)
            # pair i -> partition i%128, column i//128
            arr = np.zeros((cap_pairs, 2, H), dtype=E4M3)
            arr[:n_pairs] = sl
            bflat = np.full(cap_pairs, -1.0e9, dtype=np.float32)
            bflat[:n_pairs] = seg_rel
            bflat = bflat.astype(BF16)
            # [Cp, 128, 2, H] -> [chunk, 128, cpg, 2, H]
            a5 = arr.reshape(G, cpg, 128, 2, H).transpose(0, 2, 1, 3, 4)
            xp_c[w] = a5
            brt_c[:, w] = bflat.reshape(G, cpg, 128).transpose(2, 0, 1)
        dombT_c = np.ascontiguousarray(
            np.concatenate([dom[seg0:seg0 + spc].T,
                            np.ones((1, spc), np.float32)],
                           axis=0)).astype(m_np)
        in_maps.append({
            "xp": np.ascontiguousarray(xp_c),
            "brtA": np.ascontiguousarray(brt_c),
            "dombT": dombT_c, **shared})
    return in_maps


def _pick_cpg(batch: np.ndarray, n_cores: int) -> int:
    """Window-chunk pair-column count: max padded pair count over all
    128-segment windows, in units of 128 pairs, rounded up to G."""
    counts = np.bincount(batch, minlength=B_SEGS)
    odd = (counts % 2).astype(np.int64)
    starts = np.concatenate([[0], np.cumsum(counts)])
    pads_before = np.concatenate([[0], np.cumsum(odd)])
    pstart = starts + pads_before                      # [B+1]
    edges = pstart[::SEG_W]                            # window boundaries
    pairs = np.diff(edges) // 2
    cp = max(1, int(np.max(pairs) + 127) // 128)
    return (cp + G - 1) // G


_PROGRAM_CACHE: dict = {}

# Set by test harnesses: request an NTFF trace and stash the raw results.
TRACE = False
LAST_RESULT = None


def kernel(**inputs) -> np.ndarray:
    x = np.asarray(inputs["x"], dtype=np.float32)
    batch = np.ascontiguousarray(np.asarray(inputs["batch"]).astype(np.int64))
    assert x.shape == (N_NODES, H), x.shape

    cpg = _pick_cpg(batch, N_CORES)

    key = (cpg, N_CORES)
    if key not in _PROGRAM_CACHE:
        _PROGRAM_CACHE[key] = build_program(cpg, N_CORES)
    nc = _PROGRAM_CACHE[key]

    in_maps = prepare_core_inputs(
        x, batch,
        inputs["domain_emb"], inputs["gamma_w"], inputs["gamma_b"],
        inputs["beta_w"], inputs["beta_b"],
        inputs["w1"], inputs["b1"], inputs["w2"], inputs["b2"],
        inputs["w3"], inputs["b3"],
        cpg, N_CORES,
    )

    res = bass_utils.run_bass_kernel_spmd(
        nc, in_maps, core_ids=list(range(N_CORES)), trace=TRACE)
    global LAST_RESULT
    LAST_RESULT = res
    out = np.concatenate([res.results[c]["out"].reshape(-1) for c in range(N_CORES)])
    return np.ascontiguousarray(out.astype(np.float32))


# revision 26
# speedup vs baseline: 1.0062x; 1.0062x over previous
"""Trainium2 Bass kernel for nn_GemNetOutput (segment_reduce + FiLM + MLP head).

Reference computation (all fp32):
    g     = segment_sum(x, batch, num_segments=B)        # [B, H]
    gamma = domain_emb @ gamma_w.T + gamma_b             # [B, H]
    beta  = domain_emb @ beta_w.T  + beta_b              # [B, H]
    g     = gamma * g + beta
    h     = silu(g @ w1.T + b1)                          # [B, H]
    h     = silu(h @ w2.T + b2)                          # [B, H/2]
    out   = (h @ w3.T + b3).squeeze(-1)                  # [B]

Shapes: N=1e6 nodes, B=16384 graphs, H=512, FD=16.  `batch` is SORTED.

Strategy (8 NeuronCores, no collectives needed):
  - Shard by SEGMENT range: core c owns segments [c*2048, (c+1)*2048), i.e.
    one contiguous node slice of x (batch is sorted).  16 windows of 128
    segments per core.
  - x is quantized to fp8 E4M3 on the host with per-(segment, feature)
    ERROR DIFFUSION: q_i = rne(x_i + carry), carry += x_i - q_i.  The
    segment sum of the quantized stream telescopes to the true sum minus
    one final carry (<= ULP/2), so fp8 rounding does NOT accumulate
    sqrt(n)-style.  Odd-length segments get one pad slot that absorbs the
    final carry.  Measured end-to-end rel err ~4.8e-3 — same as a bf16-x
    scheme at ONE QUARTER the HBM traffic (512 MB total).
  - Nodes are packed into same-segment PAIRS (segments padded to even
    length).  The PE consumes a pair column per DoubleRow fp8 matmul:
    lhsT = one-hot [128, ko=2(broadcast), 128 seg], rhs = x [128, ko=2,
    512], contracting 256 nodes per 216 ns matmul — 2x the normal rate —
    and summing each pair exactly in the fp22/fp32 datapath.
  - One-hot masks are built on the DVE with a single broadcast is_equal
    tensor_tensor per window-quarter (segment-in-window ids vs an iota
    row), emitted two windows ahead so they never block the PE.
  - x DMAs are partition-contiguous QUARTER-window transfers (~1.1 MB,
    8 KB per partition).  The fine granularity keeps the PE fed every
    ~3 us so HAM never re-throttles, and releases SBUF slots at the same
    cadence so the next DMAs issue early (16-deep tile ring = 4 windows
    of prefetch).
  - FiLM runs per-window in transposed [feature, seg] layout (PE
    transpose -> DVE multiply/add).  The MLP head + gamma/beta matmuls
    are batched over PAIRS of windows (N=256) to halve their LDWEIGHTS
    overhead, and software-pipelined into the next window's DR stream.
    MLP weights/activations bf16, accumulation fp32.

Measured: 423 us (bf16 baseline) -> ~221 us, rel err ~5.9e-3.
"""

import sys
from contextlib import ExitStack

for _p in ("/opt/trn_rl_repo", "/opt/pypackages"):
    if _p not in sys.path:
        sys.path.append(_p)

import ml_dtypes
import numpy as np

import concourse.bass as bass
import concourse.tile as tile
from concourse import bacc, mybir
from concourse import bass_utils

dt = mybir.dt

# Problem constants (hardcoded per the contract).
N_NODES = 1_000_000
B_SEGS = 16_384
H = 512
H2 = 256
FD = 16
N_CORES = 8
SEG_W = 128          # segments per window (PSUM partition dim)
WINDOWS = (B_SEGS // N_CORES) // SEG_W   # 16

BF16 = ml_dtypes.bfloat16
E4M3 = ml_dtypes.float8_e4m3
E4M3_MAX = 240.0

# CoreSim has no Silu LUT; compose silu = z * sigmoid(z) when True (sim tests).
SILU_COMPOSE = False


G = 4               # DMA chunks per window


def build_program(cpg: int, n_cores: int):
    """Build the per-core Bass/Tile program.

    cpg: pair-columns per window-chunk DMA (window capacity = G * cpg
    columns x 128 pairs x 2 nodes).
    """
    spc = WINDOWS * SEG_W
    m_dt = dt.bfloat16
    x_dt = dt.float8e4
    DR = mybir.MatmulPerfMode.DoubleRow

    nc = bacc.Bacc(
        "TRN2",
        target_bir_lowering=False,
        debug=False,
        enable_asserts=False,
        num_devices=n_cores,
    )

    xp = nc.dram_tensor(
        "xp", [WINDOWS, G, 128, cpg, 2, H], x_dt, kind="ExternalInput").ap()
    brtA = nc.dram_tensor(
        "brtA", [128, WINDOWS, G, cpg], m_dt, kind="ExternalInput").ap()
    dombT = nc.dram_tensor("dombT", [FD + 1, spc], m_dt, kind="ExternalInput").ap()
    gw = nc.dram_tensor("gw", [FD + 1, H], m_dt, kind="ExternalInput").ap()
    bw = nc.dram_tensor("bw", [FD + 1, H], m_dt, kind="ExternalInput").ap()
    w1t = nc.dram_tensor("w1t", [H, H], m_dt, kind="ExternalInput").ap()
    w2t = nc.dram_tensor("w2t", [H, H2], m_dt, kind="ExternalInput").ap()
    w3c = nc.dram_tensor("w3c", [128, H2 // 128], m_dt, kind="ExternalInput").ap()
    b1c = nc.dram_tensor("b1c", [128, H // 128], dt.float32, kind="ExternalInput").ap()
    b2c = nc.dram_tensor("b2c", [128, H2 // 128], dt.float32, kind="ExternalInput").ap()
    b3c = nc.dram_tensor("b3c", [1, 1], dt.float32, kind="ExternalInput").ap()
    iden = nc.dram_tensor("iden", [128, 128], m_dt, kind="ExternalInput").ap()
    iotr = nc.dram_tensor("iotr", [128, 128], dt.float32, kind="ExternalInput").ap()
    out = nc.dram_tensor("out", [1, spc], dt.float32, kind="ExternalOutput").ap()

    HC = H // 128       # 4 h-chunks
    JC = H // 128       # 4 layer-1 output chunks
    KC = H2 // 128      # 2 layer-2 output chunks

    with tile.TileContext(nc) as tc, ExitStack() as ctx:
        cpool = ctx.enter_context(tc.tile_pool(name="consts", bufs=1))
        xpool = ctx.enter_context(tc.tile_pool(name="x", bufs=16))
        ohpool = ctx.enter_context(tc.tile_pool(name="oh", bufs=16))
        spool = ctx.enter_context(tc.tile_pool(name="work", bufs=2))
        pg = ctx.enter_context(tc.tile_pool(name="pg", bufs=2, space=bass.MemorySpace.PSUM))
        pt = ctx.enter_context(tc.tile_pool(name="pt", bufs=1, space=bass.MemorySpace.PSUM))
        pm = ctx.enter_context(tc.tile_pool(name="pm", bufs=2, space=bass.MemorySpace.PSUM))
        pgb = ctx.enter_context(tc.tile_pool(name="pgb", bufs=3, space=bass.MemorySpace.PSUM))

        # ---- constants / weights into SBUF ----
        # Order matters at the head: outstanding DMAs drain round-robin at
        # packet granularity, so everything issued before the first x chunk
        # delays it.  Tiny tiles + brt go first; the bulky MLP weights are
        # issued after window 0's x quarters.
        iden_sb = cpool.tile([128, 128], m_dt)
        nc.sync.dma_start(iden_sb[:], iden)
        iotr_sb = cpool.tile([128, 128], dt.float32)
        nc.sync.dma_start(iotr_sb[:], iotr)
        brt_sb = cpool.tile([128, WINDOWS, G, cpg], m_dt)
        nc.sync.dma_start(brt_sb[:], brtA)
        b1_sb = cpool.tile([128, JC], dt.float32)
        b2_sb = cpool.tile([128, KC], dt.float32)
        b3_sb = cpool.tile([1, 1], dt.float32)
        w3_sb = cpool.tile([128, KC], m_dt)
        w1_sb = cpool.tile([128, HC, H], m_dt)
        w2_sb = cpool.tile([128, HC, H2], m_dt)
        gw_sb = cpool.tile([FD + 1, H], m_dt)
        bw_sb = cpool.tile([FD + 1, H], m_dt)
        domT_sb = cpool.tile([FD + 1, spc], m_dt)
        out_sb = cpool.tile([1, spc], dt.float32)

        is_eq = mybir.AluOpType.is_equal

        # ---- PE warm-up: ~5us of dummy matmuls while DMA prefills, so HAM
        # flips to K=8/8 before the real stream starts.
        warm_t = pm.tile([128, H], dt.float32, tag="pmlp")
        for i in range(32):
            nc.tensor.matmul(
                warm_t[:, 0:128], iden_sb[:], iden_sb[:],
                start=(i == 0), stop=(i == 31))

        # Software pipelining: one-hot masks are built TWO windows ahead on
        # the DVE so they sit before window w's FiLM in the DVE queue (else
        # the first DR matmul of w+1 waits on transposes(w) -> FiLM(w) ->
        # oh(w+1), a ~3us PE stall per window that re-trips HAM).  x DMAs
        # and gamma/beta matmuls are likewise emitted ahead.
        oh_tiles = {}
        xt_tiles = {}
        gb_tiles = {}

        def emit_oh_dma(w):
            if w >= WINDOWS:
                return
            xts, ohs = [], []
            for jh in range(G):
                xt = xpool.tile([128, cpg, 2, H], x_dt)
                nc.sync.dma_start(xt[:], xp[w, jh])
                xts.append(xt[:])
                oh = ohpool.tile([128, cpg, 128], x_dt)
                nc.vector.tensor_tensor(
                    oh[:],
                    brt_sb[:, w, jh, :].unsqueeze(2).broadcast_to([128, cpg, 128]),
                    iotr_sb[:].unsqueeze(1).broadcast_to([128, cpg, 128]),
                    is_eq)
                ohs.append(oh)
            xt_tiles[w] = tuple(xts)
            oh_tiles[w] = tuple(ohs)

        SW2 = 2 * SEG_W

        def emit_gb_pair(k):
            # gamma/beta for window pair k (windows 2k, 2k+1): N=256 matmuls
            if 2 * k >= WINDOWS:
                return
            g_sbt = spool.tile([128, HC, 2, SEG_W], dt.float32, tag="gbg_g")
            b_sbt = spool.tile([128, HC, 2, SEG_W], dt.float32, tag="gbg_b")
            dom_s = domT_sb[:, k * SW2:(k + 1) * SW2]
            for hc in range(HC):
                for wsb, dst in ((gw_sb, g_sbt), (bw_sb, b_sbt)):
                    pgb_t = pgb.tile([128, SW2], dt.float32)
                    nc.tensor.matmul(
                        pgb_t[:],
                        wsb[:, hc * 128:(hc + 1) * 128], dom_s,
                        start=True, stop=True)
                    nc.scalar.copy(
                        dst[:, hc, :, :].rearrange("p a b -> p (a b)"), pgb_t[:])
            gb_tiles[k] = (g_sbt, b_sbt)

        def emit_dr_chunk(w, jh, pg_t):
            xt = xt_tiles[w][jh]
            oh = oh_tiles[w][jh]
            for j in range(cpg):
                nc.tensor.matmul(
                    pg_t[:],
                    oh[:, j, :].unsqueeze(1).broadcast_to([128, 2, 128]),
                    xt[:, j, :, :],
                    start=(jh == 0 and j == 0),
                    stop=(jh == G - 1 and j == cpg - 1),
                    perf_mode=DR)
            if jh == G - 1:
                xt_tiles.pop(w)
                oh_tiles.pop(w)

        def emit_mlp_l1(k, gmodT2):
            # layer 1 over a window pair: N=256 matmuls
            h1_sb = spool.tile([128, HC, SW2], m_dt, tag="h1")
            for jc in range(JC):
                ph1 = pm.tile([128, SW2], dt.float32, tag="pmlp")
                for hc in range(HC):
                    nc.tensor.matmul(
                        ph1[:],
                        w1_sb[:, hc, jc * 128:(jc + 1) * 128],
                        gmodT2[:, hc, :, :].rearrange("p a b -> p (a b)"),
                        start=(hc == 0), stop=(hc == HC - 1))
                nc.scalar.activation(
                    h1_sb[:, jc, :],
                    ph1[:],
                    mybir.ActivationFunctionType.Silu,
                    bias=b1_sb[:, jc:jc + 1])
            return h1_sb

        def emit_mlp_tail(k, h1_sb):
            h2_sb = spool.tile([128, KC, SW2], m_dt, tag="h2")
            for kc in range(KC):
                ph2 = pm.tile([128, SW2], dt.float32, tag="pmlp")
                for hc in range(HC):
                    nc.tensor.matmul(
                        ph2[:],
                        w2_sb[:, hc, kc * 128:(kc + 1) * 128],
                        h1_sb[:, hc, :],
                        start=(hc == 0), stop=(hc == HC - 1))
                nc.scalar.activation(
                    h2_sb[:, kc, :],
                    ph2[:],
                    mybir.ActivationFunctionType.Silu,
                    bias=b2_sb[:, kc:kc + 1])
            po = pm.tile([1, SW2], dt.float32, tag="pmlp")
            for kc in range(KC):
                nc.tensor.matmul(
                    po[:], w3_sb[:, kc:kc + 1],
                    h2_sb[:, kc, :],
                    start=(kc == 0), stop=(kc == KC - 1))
            nc.scalar.activation(
                out_sb[0:1, k * SW2:(k + 1) * SW2], po[:],
                mybir.ActivationFunctionType.Identity,
                bias=b3_sb[0:1, 0:1])
            nc.sync.dma_start(
                out[0:1, k * SW2:(k + 1) * SW2],
                out_sb[0:1, k * SW2:(k + 1) * SW2])

        emit_oh_dma(0)
        # everything not needed for window 0's DR stream is issued after
        # its x quarters
        nc.sync.dma_start(b1_sb[:], b1c)
        nc.sync.dma_start(b2_sb[:], b2c)
        nc.sync.dma_start(b3_sb[:], b3c)
        nc.sync.dma_start(w3_sb[:], w3c)
        nc.sync.dma_start(gw_sb[:], gw)
        nc.sync.dma_start(bw_sb[:], bw)
        nc.sync.dma_start(domT_sb[:], dombT)
        emit_oh_dma(1)
        nc.sync.dma_start(w1_sb[:], w1t.rearrange("(c p) j -> p c j", p=128))
        nc.sync.dma_start(w2_sb[:], w2t.rearrange("(c p) j -> p c j", p=128))
        emit_gb_pair(0)

        # Software pipeline: pair k's MLP (N=256 over both windows) is
        # interleaved into window 2k+2's DR stream, so the PE never waits
        # on the evict->transpose->FiLM chain or the silu round-trips.
        film = {}       # k -> gmodT2 tile (both windows of the pair)
        h1s = {}        # k -> h1 tile
        for w in range(WINDOWS):
            k, half = divmod(w, 2)
            pg_t = pg.tile([128, H], dt.float32)
            emit_dr_chunk(w, 0, pg_t)
            emit_oh_dma(w + 2)
            emit_dr_chunk(w, 1, pg_t)
            if half == 0 and k >= 1:
                h1s[k - 1] = emit_mlp_l1(k - 1, film.pop(k - 1))
            emit_dr_chunk(w, 2, pg_t)
            emit_dr_chunk(w, 3, pg_t)
            if half == 0 and k >= 1:
                emit_mlp_tail(k - 1, h1s.pop(k - 1))
            # evict g early, in h-chunks: the first transpose can start
            # after only 1/4 of the eviction
            g_sb = spool.tile([128, H], m_dt, tag="g")
            for hc in range(HC):
                nc.scalar.copy(
                    g_sb[:, hc * 128:(hc + 1) * 128],
                    pg_t[:, hc * 128:(hc + 1) * 128])
            if half == 1:
                emit_gb_pair(k + 1)

            # transpose + FiLM for this window (into the pair tile)
            g_sbt, b_sbt = gb_tiles[k]
            pt_t = pt.tile([128, H], m_dt)
            for hc in range(HC):
                nc.tensor.transpose(
                    pt_t[:, hc * 128:(hc + 1) * 128],
                    g_sb[:, hc * 128:(hc + 1) * 128],
                    iden_sb[:])
            if half == 0:
                gmodT2 = spool.tile([128, HC, 2, SEG_W], m_dt, tag="gmodT")
                film[k] = gmodT2
            else:
                gmodT2 = film[k]
            pt_v = pt_t[:].rearrange("p (c s) -> p c s", c=HC)
            gm_v = gmodT2[:, :, half, :]
            nc.vector.tensor_mul(gm_v, pt_v, g_sbt[:, :, half, :])
            nc.vector.tensor_add(gm_v, gm_v, b_sbt[:, :, half, :])
            if half == 1:
                gb_tiles.pop(k)

        kl = WINDOWS // 2 - 1
        h1s[kl] = emit_mlp_l1(kl, film.pop(kl))
        emit_mlp_tail(kl, h1s.pop(kl))

    nc.compile()
    return nc


def diffuse_quantize(x: np.ndarray, counts: np.ndarray, starts: np.ndarray):
    """Error-diffusion quantization of x to E4M3, sequential within each
    segment (vectorized over segments x features).  Returns the quantized
    bytes for every node plus, for odd-length segments, a pad value that
    absorbs the final carry."""
    B = len(counts)
    nH = x.shape[1]
    qx = np.empty(x.shape, dtype=E4M3)
    carry = np.zeros((B, nH), np.float32)
    maxn = int(counts.max()) if B else 0
    for k in range(maxn):
        active = np.nonzero(counts > k)[0]
        if len(active) == 0:
            break
        idx = starts[active] + k
        v = x[idx] + carry[active]
        q = np.clip(v, -E4M3_MAX, E4M3_MAX).astype(E4M3)
        qx[idx] = q
        carry[active] = v - q.astype(np.float32)
    odd = np.nonzero((counts % 2 == 1) & (counts > 0))[0]
    pad_q = np.zeros((B, nH), dtype=E4M3)
    if len(odd):
        pad_q[odd] = np.clip(carry[odd], -E4M3_MAX, E4M3_MAX).astype(E4M3)
    return qx, pad_q


def prepare_core_inputs(
    x, batch, domain_emb, gamma_w, gamma_b, beta_w, beta_b,
    w1, b1, w2, b2, w3, b3,
    cpg: int, n_cores: int,
):
    """Quantize, pad, pack and transpose the full inputs into one in_map
    per core."""
    spc = B_SEGS // n_cores
    Cp = G * cpg
    cap_pairs = 128 * Cp

    batch = np.ascontiguousarray(np.asarray(batch).astype(np.int64))
    x = np.asarray(x, dtype=np.float32)
    n = x.shape[0]

    counts = np.bincount(batch, minlength=B_SEGS)
    starts = np.concatenate([[0], np.cumsum(counts)])[:B_SEGS]

    qx, pad_q = diffuse_quantize(x, counts, starts)

    # --- build the padded per-segment stream (pairs stay within-segment
    # because every padded run has even length) ---
    odd = (counts % 2).astype(np.int64)
    pads_before = np.concatenate([[0], np.cumsum(odd)])[:B_SEGS]
    pstart = starts + pads_before                       # stream offset per segment
    m_total = int(n + odd.sum())
    pstart_full = np.concatenate([pstart, [m_total]])

    stream = np.zeros((m_total, H), dtype=E4M3)
    node_pos = np.arange(n, dtype=np.int64) + pads_before[batch]
    stream[node_pos] = qx
    stream_seg = np.zeros(m_total, dtype=np.int64)
    stream_seg[node_pos] = batch
    odd_segs = np.nonzero(odd)[0]
    if len(odd_segs):
        pad_pos = pstart[odd_segs] + counts[odd_segs]
        stream[pad_pos] = pad_q[odd_segs]
        stream_seg[pad_pos] = odd_segs

    m_np = BF16
    shared = {
        "gw": np.ascontiguousarray(
            np.concatenate([np.asarray(gamma_w, np.float32).T,
                            np.asarray(gamma_b, np.float32)[None]],
                           axis=0)).astype(m_np),
        "bw": np.ascontiguousarray(
            np.concatenate([np.asarray(beta_w, np.float32).T,
                            np.asarray(beta_b, np.float32)[None]],
                           axis=0)).astype(m_np),
        "w1t": np.ascontiguousarray(np.asarray(w1, np.float32).T.astype(m_np)),
        "w2t": np.ascontiguousarray(np.asarray(w2, np.float32).T.astype(m_np)),
        "w3c": np.ascontiguousarray(
            np.asarray(w3, np.float32).reshape(H2 // 128, 128).T.astype(m_np)),
        "b1c": np.ascontiguousarray(np.asarray(b1, np.float32).reshape(H // 128, 128).T),
        "b2c": np.ascontiguousarray(np.asarray(b2, np.float32).reshape(H2 // 128, 128).T),
        "b3c": np.asarray(b3, np.float32).reshape(1, 1),
        "iden": np.eye(128, dtype=np.float32).astype(m_np),
        "iotr": np.tile(np.arange(128, dtype=np.float32), (128, 1)),
    }

    dom = np.asarray(domain_emb, np.float32)

    in_maps = []
    for core in range(n_cores):
        seg0 = core * spc
        xp_c = np.zeros((WINDOWS, G, 128, cpg, 2, H), dtype=E4M3)
        brt_c = np.full((128, WINDOWS, G, cpg), -1.0e9, dtype=BF16)
        for w in range(WINDOWS):
            s_lo = seg0 + w * SEG_W
            lo = int(pstart_full[s_lo])
            hi = int(pstart_full[s_lo + SEG_W])
            n_pairs = (hi - lo) // 2
            if n_pairs == 0:
                continue
            if n_pairs > cap_pairs:
                raise ValueError(f"window overflow: {n_pairs} > {cap_pairs}")
            sl = stream[lo:hi].reshape(n_pairs, 2, H)
            seg_rel = (stream_seg[lo:hi:2] - s_lo).astype(np.float32)
            # pair i -> partition i%128, column i//128
            arr = np.zeros((cap_pairs, 2, H), dtype=E4M3)
            arr[:n_pairs] = sl
            bflat = np.full(cap_pairs, -1.0e9, dtype=np.float32)
            bflat[:n_pairs] = seg_rel
            bflat = bflat.astype(BF16)
            # [Cp, 128, 2, H] -> [chunk, 128, cpg, 2, H]
            a5 = arr.reshape(G, cpg, 128, 2, H).transpose(0, 2, 1, 3, 4)
            xp_c[w] = a5
            brt_c[:, w] = bflat.reshape(G, cpg, 128).transpose(2, 0, 1)
        dombT_c = np.ascontiguousarray(
            np.concatenate([dom[seg0:seg0 + spc].T,
                            np.ones((1, spc), np.float32)],
                           axis=0)).astype(m_np)
        in_maps.append({
            "xp": np.ascontiguousarray(xp_c),
            "brtA": np.ascontiguousarray(brt_c),
            "dombT": dombT_c, **shared})
    return in_maps


def _pick_cpg(batch: np.ndarray, n_cores: int) -> int:
    """Window-chunk pair-column count: max padded pair count over all
    128-segment windows, in units of 128 pairs, rounded up to G."""
    counts = np.bincount(batch, minlength=B_SEGS)
    odd = (counts % 2).astype(np.int64)
    starts = np.concatenate([[0], np.cumsum(counts)])
    pads_before = np.concatenate([[0], np.cumsum(odd)])
    pstart = starts + pads_before                      # [B+1]
    edges = pstart[::SEG_W]                            # window boundaries
    pairs = np.diff(edges) // 2
    cp = max(1, int(np.max(pairs) + 127) // 128)
    return (cp + G - 1) // G


_PROGRAM_CACHE: dict = {}

# Set by test harnesses: request an NTFF trace and stash the raw results.
TRACE = False
LAST_RESULT = None


def kernel(**inputs) -> np.ndarray:
    x = np.asarray(inputs["x"], dtype=np.float32)
    batch = np.ascontiguousarray(np.asarray(inputs["batch"]).astype(np.int64))
    assert x.shape == (N_NODES, H), x.shape

    cpg = _pick_cpg(batch, N_CORES)

    key = (cpg, N_CORES)
    if key not in _PROGRAM_CACHE:
        _PROGRAM_CACHE[key] = build_program(cpg, N_CORES)
    nc = _PROGRAM_CACHE[key]

    in_maps = prepare_core_inputs(
        x, batch,
        inputs["domain_emb"], inputs["gamma_w"], inputs["gamma_b"],
        inputs["beta_w"], inputs["beta_b"],
        inputs["w1"], inputs["b1"], inputs["w2"], inputs["b2"],
        inputs["w3"], inputs["b3"],
        cpg, N_CORES,
    )

    res = bass_utils.run_bass_kernel_spmd(
        nc, in_maps, core_ids=list(range(N_CORES)), trace=TRACE)
    global LAST_RESULT
    LAST_RESULT = res
    out = np.concatenate([res.results[c]["out"].reshape(-1) for c in range(N_CORES)])
    return np.ascontiguousarray(out.astype(np.float32))
